# revision 32
# baseline (speedup 1.0000x reference)
"""Multi-head attention (B=2, N=4096, C=768, H=12, D=64) on 8 TRN2 NeuronCores.

Sharding: tensor-parallel over (batch, head). B*H = 24 pairs -> 3 per core.
Cores 0-3 handle batch 0, cores 4-7 batch 1 (3 consecutive heads each).
Each core computes the QKV projection, attention, and a partial output
projection for its heads, returning a partial y^T [768, 4096] in bf16.
The host sums the 4 partials per batch in fp32, transposes, adds the bias.

All PE operands are bf16 (fp32 PSUM accumulation). The fp32r variant of
this kernel spent 70% of its time power-throttled to 1.2 GHz; bf16 halves
the SBUF streaming bandwidth and MAC energy and runs ~95% unthrottled.

The scalar engine is the structural floor: 50.3M softmax exps per core
at 1 elem/lane/cycle (~365us busy incl. per-instruction overhead); the
PE (MM1+MM2 stream 786k PSUM columns plus projections) sits just under
it at ~84% occupancy. Everything is scheduled to keep both saturated:

- Each head owns a fixed SBUF partition home -- h0 at 0:64, h1 at 64:128,
  h2 at 0:64 -- matching how the host packs the projection weights
  ([q0|q1], [k0|k1], [q2|k2]), so phase A unpacks PSUM with plain
  same-partition copies (one SBUF->SBUF DMA per slice relocates k2).
  MM1 for h1 runs at PE tile position (64,0).
- The attention for (j0,h0) and (j0,h1) is interleaved chunk-by-chunk
  into phase A (1-chunk exp groups), so the scalar engine works from
  ~23us in. This also smooths the power ramp that previously drew a
  133us half-clock throttle window at the phase A->C transition.
- MM1 alternates even k-chunks onto PE row-group half 0:64 and odd
  chunks onto 64:128 (via partition-swapped copies of q^T/k^T built with
  two SBUF->SBUF DMAs per slice): consecutive MM1s never share a row
  group, so the PE reorder window pulls every LDWEIGHTS ahead of the
  in-flight matmul and adjacent MM1s partially overlap (248 -> 183 ns
  measured for N=512).
- MM2 groups are emitted up to TWO exp-groups late: in the strictly
  in-order PE queue, the next MM1 trio must never sit behind an MM2
  backlog waiting on exp, or the scalar (bottleneck) goes idle -- the
  depth-2 queue keeps head boundaries seamless. The same deferral runs
  continuously across heads and q-slices; the output projection is
  deferred by one q-slice.
- exp runs on ScalarE straight from PSUM [128,1536] (3 banks, double
  buffered; PSUM budget is exactly 8 banks with o_t + y PSUM) into bf16
  SBUF. Scores are O(2) by construction; max-subtraction is unnecessary.
- v_aug slots are 128 wide ([64 v | ones col | 63 zero pad]) so MM2's
  weight loads take the fast-weight-load path; the ones column
  accumulates softmax denominators in PSUM row 64 for free.
- normalize runs off the PE: the [1,512] denominator row is reshaped to
  [128,4] by DMA so the iterative-divide DVE reciprocal costs 32 cycles
  instead of 4096, then a DRAM round-trip DMA broadcasts 1/d across 64
  partitions (partition-stride-0 read) for one DVE multiply per head.

Rejected experiments (measured slower or too lossy on HW): fp8 DoubleRow
MM1 with k hi+lo expansion (LDWEIGHTS-bound, 545us and 1.8e-2 error),
PE row-tiling paired MM1 chunks (2nd tile streams +163ns late, no gain),
per-chunk prologue DMA splitting (HW race -> wrong results).
"""

import numpy as np
import ml_dtypes

import concourse.bass as bass
import concourse.mybir as mybir
import concourse.tile as tile
from concourse import bacc
from concourse.bass_utils import run_bass_kernel_spmd

F32 = mybir.dt.float32
BF16 = mybir.dt.bfloat16

DIM = 768
NUM_HEADS = 12
HEAD_DIM = 64
SCALE = HEAD_DIM ** -0.5
B = 2
N_FULL = 4096
N_CORES = 8
HEADS_PER_CORE = 3
CC = DIM // 128  # 6 contraction chunks


def build_nc(n=N_FULL, fast_mm=True):
    """Build the per-core Bass program. Same program runs SPMD on all
    cores; per-core inputs differ (x^T batch + per-head weight slices)."""
    del fast_mm
    nj = n // 512      # q slices
    nk = n // 128      # k chunks
    nc = bacc.Bacc("TRN2", target_bir_lowering=False, debug=False)

    # x^T and weights pre-tiled on host to partition-major layouts so DMA
    # lines are contiguous multi-KB runs per partition.
    xt_d = nc.dram_tensor("xt", [128, nj, CC, 512], BF16, kind="ExternalInput")
    wqk_d = nc.dram_tensor("wqk", [128, CC, 384], BF16, kind="ExternalInput")
    wv_d = nc.dram_tensor("wv", [128, CC, 192], BF16, kind="ExternalInput")
    wp_d = nc.dram_tensor("wp", [192, DIM], BF16, kind="ExternalInput")
    # partial y^T in bf16: host sums the four per-core partials in fp32
    yt_d = nc.dram_tensor("yt", [DIM, n], BF16, kind="ExternalOutput")

    # MM1/exp/MM2 group sizes: 3 k-chunks (3 PSUM banks) per exp.
    groups = [3] * (nk // 3)
    if nk % 3:
        groups.append(nk % 3)

    lp = nc.allow_low_precision(
        reason="bf16 matmul operands; PSUM accumulation stays fp32")
    with lp, tile.TileContext(nc) as tc:
        consts = tc.alloc_tile_pool(name="consts", bufs=1)
        persist = tc.alloc_tile_pool(name="persist", bufs=1)

        wqk_sb = consts.tile([128, CC, 384], BF16, tag="wqk")
        wv_sb = consts.tile([128, CC, 192], BF16, tag="wv")

        # Preload the exp table set (~2.7us) off the critical path.
        warm = consts.tile([1, 4], F32, tag="warm")
        nc.vector.memset(warm, 0.0)
        nc.scalar.activation(out=warm, in_=warm,
                             func=mybir.ActivationFunctionType.Exp)

        # Persistent activations. Partition homes: h0 0:64, h1 64:128,
        # h2 0:64; the *x tensors hold the same data with the partition
        # halves swapped. MM1 sends even k-chunks to one PE row-group half
        # and odd chunks to the other: consecutive matmuls then never
        # share a row group, so the PE's reorder window can pull each
        # LDWEIGHTS ahead of the in-flight matmul (~35ns/matmul).
        qt01 = persist.tile([128, n], BF16, tag="qt01")
        qt01x = persist.tile([128, n], BF16, tag="qt01x")
        kt01 = persist.tile([128, nk, 128], BF16, tag="kt01")
        kt01x = persist.tile([128, nk, 128], BF16, tag="kt01x")
        qt2 = persist.tile([128, n], BF16, tag="qt2")
        kt2 = persist.tile([128, nk, 128], BF16, tag="kt2")
        # v_aug slot: [64 v dims | ones | 63 zero pad] -> 128-wide weights
        # keep MM2 on the fast-weight-load path.
        v_aug = persist.tile([128, HEADS_PER_CORE, nk, 128], BF16, tag="vaug")
        # (j0,h2) softmax rows computed during phase A (exp needs no PSUM
        # O-accumulator bank, so a third head CAN overlap exp-only); its
        # MM2s drip into early phase C where the PE has slack.
        pt2_store = persist.tile([128, nk, 512], BF16, tag="pt2")
        nc.vector.memset(v_aug[:, :, :, 64:65], 1.0)
        nc.vector.memset(v_aug[:, :, :, 65:128], 0.0)

        # Pools shared by the overlapped phase-A attention prefix and
        # phase C. PSUM budget: o_ps 2 banks + phase A (qk 2, v 2, s_mini 2)
        # = 8; after phase A closes, s_ps (6 banks) + o_ps = 8.
        wp_sb = {}
        with (
            tc.tile_pool(name="o_ps", bufs=2, space="PSUM") as o_ps,
            tc.tile_pool(name="dscr", bufs=3, space="DRAM") as dscr_p,
            tc.tile_pool(name="ptp", bufs=6) as ptp,
            tc.tile_pool(name="otp", bufs=4) as otp,
            tc.tile_pool(name="rsbp", bufs=3) as rsbp,
            tc.tile_pool(name="ytp", bufs=4) as ytp,
        ):
            ots_by_j = {}
            pending_proj = None
            state = {"pend": []}
            o_ts = {}

            def emit_outproj(pj, pots):
                # pots = [ot01 (parts 0:64 h0, 64:128 h1), ot2]
                pjsl = bass.ts(pj, 512)
                ot01, ot2 = pots
                for cc in range(CC):
                    yps = o_ps.tile([128, 512], F32, tag="o", name="yps")
                    nc.tensor.matmul(
                        yps, wp_sb["01"][:, bass.ts(cc, 128)], ot01,
                        start=True, stop=False,
                    )
                    nc.tensor.matmul(
                        yps, wp_sb["2"][:, bass.ts(cc, 128)], ot2,
                        start=False, stop=True,
                    )
                    yst = ytp.tile([128, 512], BF16, tag="yt")
                    nc.vector.tensor_copy(out=yst, in_=yps)
                    nc.sync.dma_start(out=yt_d[bass.ts(cc, 128), pjsl], in_=yst)

            def normalize(pj, ph, o_t):
                nonlocal pending_proj
                # denominators: [1,512] PSUM row -> [128,4] via DMA so the
                # 8-cycle/element reciprocal runs 32 cycles, then a DRAM
                # round trip broadcasts 1/d to 64 partitions.
                rsb = rsbp.tile([128, 512], F32, tag="r")
                nc.vector.tensor_copy(out=rsb[64:65, :], in_=o_t[64:65, :])
                d4 = rsbp.tile([128, 4], F32, tag="d4")
                nc.sync.dma_start(out=d4, in_=rsb[64:65, :])
                r4 = rsbp.tile([128, 4], F32, tag="r4")
                nc.vector.reciprocal(out=r4, in_=d4)
                scr = dscr_p.tile([512], F32, tag="scr")
                nc.sync.dma_start(
                    out=scr.rearrange("(p i) -> p i", p=128), in_=r4)
                bcs = rsbp.tile([64, 512], F32, tag="bcs")
                scr_b = bass.AP(tensor=scr.tensor, offset=scr.offset,
                                ap=[[0, 64]] + list(scr.ap))
                nc.sync.dma_start(out=bcs, in_=scr_b)
                ots = ots_by_j.setdefault(pj, [])
                if ph == 0:
                    ot01 = otp.tile([128, 512], BF16, tag="ot01", name="ot01")
                    nc.vector.tensor_mul(ot01[0:64, :], o_t[0:64, :], bcs)
                    ots.append(ot01)
                    if pending_proj is not None:
                        emit_outproj(*pending_proj)
                        pending_proj = None
                elif ph == 1:
                    ot1 = otp.tile([64, 512], BF16, tag="ot1", name="ot1")
                    nc.vector.tensor_mul(ot1, o_t[0:64, :], bcs)
                    nc.sync.dma_start(out=ots[0][64:128, :], in_=ot1)
                else:
                    ot2 = otp.tile([64, 512], BF16, tag="ot2", name="ot2")
                    nc.vector.tensor_mul(ot2, o_t[0:64, :], bcs)
                    ots.append(ot2)
                    pending_proj = (pj, ots)

            def flush_mm2():
                pj, ph, ks_p, gs_p, ptt_p = state["pend"].pop(0)
                key = (pj, ph)
                if key not in o_ts:
                    o_ts[key] = o_ps.tile([128, 512], F32, tag="o", name="o_t")
                o_t = o_ts[key]
                for t in range(gs_p):
                    kc = ks_p + t
                    nc.tensor.matmul(
                        o_t, v_aug[:, ph, kc, :],
                        ptt_p[:, bass.ts(t, 512)],
                        start=(kc == 0), stop=(kc == nk - 1),
                    )
                if ks_p + gs_p == nk:  # head complete
                    del o_ts[key]
                    normalize(pj, ph, o_t)

            def mm1_operands(pj, ph, kc):
                pjsl = bass.ts(pj, 512)
                if ph == 0:
                    if kc % 2 == 0:
                        return kt01[0:64, kc, :], qt01[0:64, pjsl]
                    return kt01x[64:128, kc, :], qt01x[64:128, pjsl]
                if ph == 1:
                    if kc % 2 == 0:
                        return kt01[64:128, kc, :], qt01[64:128, pjsl]
                    return kt01x[0:64, kc, :], qt01x[0:64, pjsl]
                if kc % 2 == 0:
                    return kt2[0:64, kc, :], qt2[0:64, pjsl]
                return kt2[64:128, kc, :], qt2[64:128, pjsl]

            # ---- Phase A: QKV projections, with the attention for
            # (j0,h0)/(j0,h1) interleaved chunk-by-chunk so the scalar
            # engine (the kernel bottleneck) is busy from the start ----
            with (
                tc.tile_pool(name="xtj", bufs=3) as xtj_p,
                tc.tile_pool(name="stage", bufs=2) as stage_p,
                tc.tile_pool(name="qk_ps", bufs=2, space="PSUM") as qk_ps,
                tc.tile_pool(name="v_ps", bufs=2, space="PSUM") as v_ps,
                tc.tile_pool(name="s_mini", bufs=2, space="PSUM") as s_mini,
            ):
                for j in range(nj):
                    jsl = bass.ts(j, 512)
                    xtj = xtj_p.tile([128, CC, 512], BF16, tag="xtj")
                    nc.sync.dma_start(out=xtj, in_=xt_d[:, j])
                    if j == 0:
                        nc.sync.dma_start(out=wqk_sb, in_=wqk_d[:, :, :])
                        nc.sync.dma_start(out=wv_sb, in_=wv_d[:, :, :])
                    # q/k projections: [q0|q1], [k0|k1], [q2|k2]
                    for pi, colbase in enumerate((0, 128, 256)):
                        ps = qk_ps.tile([128, 512], F32, tag="qk")
                        for cc in range(CC):
                            nc.tensor.matmul(
                                ps,
                                wqk_sb[:, cc, colbase:colbase + 128],
                                xtj[:, cc, :],
                                start=(cc == 0), stop=(cc == CC - 1),
                            )
                        if pi == 0:
                            nc.vector.tensor_copy(out=qt01[:, jsl], in_=ps)
                            nc.sync.dma_start(out=qt01x[64:128, jsl],
                                              in_=qt01[0:64, jsl])
                            nc.sync.dma_start(out=qt01x[0:64, jsl],
                                              in_=qt01[64:128, jsl])
                        elif pi == 1:
                            nc.vector.tensor_copy(
                                out=kt01[:, 4 * j:4 * j + 4, :], in_=ps)
                            nc.sync.dma_start(
                                out=kt01x[64:128, 4 * j:4 * j + 4, :],
                                in_=kt01[0:64, 4 * j:4 * j + 4, :])
                            nc.sync.dma_start(
                                out=kt01x[0:64, 4 * j:4 * j + 4, :],
                                in_=kt01[64:128, 4 * j:4 * j + 4, :])
                        else:
                            # q2 -> partitions 0:64 + DMA-duplicated to the
                            # upper half; k2 odd chunks land at 64:128
                            # directly, even chunks relocate down via DMA
                            nc.vector.tensor_copy(out=qt2[0:64, jsl], in_=ps[0:64, :])
                            nc.sync.dma_start(out=qt2[64:128, jsl],
                                              in_=qt2[0:64, jsl])
                            nc.vector.tensor_copy(
                                out=kt2[64:128, 4 * j + 1, :],
                                in_=ps[64:128, 128:256])
                            nc.vector.tensor_copy(
                                out=kt2[64:128, 4 * j + 3, :],
                                in_=ps[64:128, 384:512])
                            st = stage_p.tile([128, 512], BF16, tag="st")
                            nc.vector.tensor_copy(out=st[64:128, 0:128],
                                                  in_=ps[64:128, 0:128])
                            nc.vector.tensor_copy(out=st[64:128, 128:256],
                                                  in_=ps[64:128, 256:384])
                            nc.sync.dma_start(out=kt2[0:64, 4 * j, :],
                                              in_=st[64:128, 0:128])
                            nc.sync.dma_start(out=kt2[0:64, 4 * j + 2, :],
                                              in_=st[64:128, 128:256])

                    # v projection (natural orientation), 3 heads packed
                    for rc in range(4):
                        psv = v_ps.tile([128, 192], F32, tag="v")
                        for cc in range(CC):
                            nc.tensor.matmul(
                                psv,
                                xtj[:, cc, bass.ts(rc, 128)],
                                wv_sb[:, cc, :],
                                start=(cc == 0), stop=(cc == CC - 1),
                            )
                        kc = j * 4 + rc
                        for h in range(HEADS_PER_CORE):
                            nc.vector.tensor_copy(
                                out=v_aug[:, h, kc, 0:64], in_=psv[:, bass.ts(h, 64)]
                            )

                    # attention prefix: chunks 4j..4j+3 of (j0,h0..h2);
                    # h0/h1 run MM1+exp+MM2, h2 runs MM1+exp only (no third
                    # PSUM O bank exists) with P stashed for phase C
                    for c in range(4 * j, 4 * j + 4):
                        for h in (0, 1, 2):
                            kt_sl, q_sl = mm1_operands(0, h, c)
                            sp = s_mini.tile([128, 512], F32, tag="sm")
                            nc.tensor.matmul(sp, kt_sl, q_sl,
                                             start=True, stop=True)
                            if h == 2:
                                nc.scalar.activation(
                                    out=pt2_store[:, c, :], in_=sp,
                                    func=mybir.ActivationFunctionType.Exp,
                                )
                                continue
                            ptt = ptp.tile([128, 512], BF16, tag="pt1",
                                           name="pt1")
                            nc.scalar.activation(
                                out=ptt, in_=sp,
                                func=mybir.ActivationFunctionType.Exp,
                            )
                            if len(state["pend"]) >= 2:
                                flush_mm2()
                            state["pend"].append((0, h, c, 1, ptt))

            # out-projection weights are first needed one q-slice into
            # phase C; loading them late keeps the prologue DMA clear.
            wp_sb["01"] = consts.tile([128, DIM], BF16, tag="wp01", name="wp01")
            nc.sync.dma_start(out=wp_sb["01"], in_=wp_d[0:128, :])
            wp_sb["2"] = consts.tile([64, DIM], BF16, tag="wp2", name="wp2")
            nc.sync.dma_start(out=wp_sb["2"], in_=wp_d[128:192, :])

            # ---- Phase C/D: remaining heads + output projection ----
            head_list = [(j, h) for j in range(1, nj)
                         for h in range(HEADS_PER_CORE)]
            drip = {"next": 0, "o_t": None}

            def drip_h2_mm2():
                # three chunks of (j0,h2) MM2 from the stashed P rows
                kc0 = drip["next"]
                if kc0 >= nk:
                    return
                if drip["o_t"] is None:
                    drip["o_t"] = o_ps.tile([128, 512], F32, tag="o",
                                            name="o_t2")
                take = min(4, nk - kc0)
                for kc in range(kc0, kc0 + take):
                    nc.tensor.matmul(
                        drip["o_t"], v_aug[:, 2, kc, :], pt2_store[:, kc, :],
                        start=(kc == 0), stop=(kc == nk - 1),
                    )
                drip["next"] = kc0 + take
                if drip["next"] >= nk:
                    normalize(0, 2, drip["o_t"])

            gi_global = 0
            with tc.tile_pool(name="s_ps", bufs=2, space="PSUM") as s_ps:
                for j, h in head_list:
                    ks = 0
                    for gsize in groups:
                        sp = s_ps.tile([128, 1536], F32, tag="s")
                        for t in range(gsize):
                            kt_sl, q_sl = mm1_operands(j, h, ks + t)
                            nc.tensor.matmul(
                                sp[:, bass.ts(t, 512)], kt_sl,
                                q_sl, start=True, stop=True,
                            )
                        ptt = ptp.tile([128, 1536], BF16, tag="pt")
                        nc.scalar.activation(
                            out=ptt[:, 0:gsize * 512], in_=sp[:, 0:gsize * 512],
                            func=mybir.ActivationFunctionType.Exp,
                        )
                        # MM2 up to two groups behind: in the in-order PE
                        # queue, exp(g) must never wait on a trio that sits
                        # behind an MM2 backlog.
                        if len(state["pend"]) >= 2:
                            flush_mm2()
                        # delay the (0,2) drip two groups so o_t buffers of
                        # (0,0)/(0,1) are released first (2-bank pool)
                        if gi_global >= 2:
                            drip_h2_mm2()
                        gi_global += 1
                        state["pend"].append((j, h, ks, gsize, ptt))
                        ks += gsize
                while state["pend"]:
                    flush_mm2()
                while drip["next"] < nk:
                    drip_h2_mm2()
                emit_outproj(*pending_proj)

        persist.release()
        consts.release()

    nc.compile()
    return nc


def make_core_inputs(x_b, w_qkv, w_proj, hbase):
    """Per-core input arrays for heads [hbase, hbase+3) of batch x_b."""
    C = DIM
    bf16 = ml_dtypes.bfloat16
    wq = [w_qkv[(hbase + h) * 64:(hbase + h + 1) * 64, :] * SCALE for h in range(3)]
    wk = [w_qkv[C + (hbase + h) * 64:C + (hbase + h + 1) * 64, :] for h in range(3)]
    wv = [w_qkv[2 * C + (hbase + h) * 64:2 * C + (hbase + h + 1) * 64, :] for h in range(3)]

    wqk = np.zeros((C, 384), np.float32)
    wqk[:, 0:64] = wq[0].T
    wqk[:, 64:128] = wq[1].T
    wqk[:, 128:192] = wk[0].T
    wqk[:, 192:256] = wk[1].T
    wqk[:, 256:320] = wq[2].T
    wqk[:, 320:384] = wk[2].T
    # pre-tile [768, m] -> [p, a, m] (row a*128+p) for contiguous DMA lines
    wqk = np.ascontiguousarray(wqk.reshape(CC, 128, 384).transpose(1, 0, 2))

    wv_p = np.zeros((C, 192), np.float32)
    for h in range(3):
        wv_p[:, h * 64:(h + 1) * 64] = wv[h].T
    wv_p = np.ascontiguousarray(wv_p.reshape(CC, 128, 192).transpose(1, 0, 2))

    wp = np.zeros((192, C), np.float32)
    for h in range(3):
        wp[h * 64:(h + 1) * 64, :] = w_proj[:, (hbase + h) * 64:(hbase + h + 1) * 64].T

    # x^T [768, n] -> [p, j, a, m] tiling: row a*128+p, col j*512+m
    n = x_b.shape[0]
    xt = np.ascontiguousarray(
        x_b.T.reshape(CC, 128, n // 512, 512).transpose(1, 2, 0, 3))
    return {
        "xt": xt.astype(bf16),
        "wqk": wqk.astype(bf16),
        "wv": wv_p.astype(bf16),
        "wp": wp.astype(bf16),
    }


_NC_CACHE = {}


def get_nc(n=N_FULL, fast_mm=True):
    key = (n, fast_mm)
    if key not in _NC_CACHE:
        _NC_CACHE[key] = build_nc(n, fast_mm)
    return _NC_CACHE[key]


def kernel(x, w_qkv, w_proj, b_proj, _trace=False):
    x = np.asarray(x, np.float32)
    w_qkv = np.asarray(w_qkv, np.float32)
    w_proj = np.asarray(w_proj, np.float32)
    b_proj = np.asarray(b_proj, np.float32)

    nc = get_nc(N_FULL, True)
    in_maps = []
    for c in range(N_CORES):
        b = c // 4
        hbase = (c % 4) * HEADS_PER_CORE
        in_maps.append(make_core_inputs(x[b], w_qkv, w_proj, hbase))

    res = run_bass_kernel_spmd(nc, in_maps, core_ids=list(range(N_CORES)),
                               trace=_trace)
    y = np.empty((B, N_FULL, DIM), np.float32)
    for b in range(B):
        acc = res.results[4 * b]["yt"].astype(np.float32)
        for c in range(4 * b + 1, 4 * b + 4):
            acc = acc + res.results[c]["yt"]
        y[b] = acc.T + b_proj[None, :]
    if _trace:
        return y, res
    return y


# revision 34
# speedup vs baseline: 1.1814x; 1.1814x over previous
"""Multi-head attention (B=2, N=4096, C=768, H=12, D=64) on 8 TRN2 NeuronCores.

Sharding: tensor-parallel over (batch, head). B*H = 24 pairs -> 3 per core.
Cores 0-3 handle batch 0, cores 4-7 batch 1 (3 consecutive heads each).
Each core computes the QKV projection, attention, and a partial output
projection for its heads, returning a partial y^T [768, 4096] in bf16.
The host sums the 4 partials per batch in fp32, transposes, adds the bias.

All PE operands are bf16 (fp32 PSUM accumulation). The fp32r variant of
this kernel spent 70% of its time power-throttled to 1.2 GHz; bf16 halves
the SBUF streaming bandwidth and MAC energy and runs ~95% unthrottled.

The scalar engine is the structural floor: 50.3M softmax exps per core
at 1 elem/lane/cycle (~365us busy incl. per-instruction overhead); the
PE (MM1+MM2 stream 786k PSUM columns plus projections) sits just under
it at ~84% occupancy. Everything is scheduled to keep both saturated:

- Each head owns a fixed SBUF partition home -- h0 at 0:64, h1 at 64:128,
  h2 at 0:64 -- matching how the host packs the projection weights
  ([q0|q1], [k0|k1], [q2|k2]), so phase A unpacks PSUM with plain
  same-partition copies (one SBUF->SBUF DMA per slice relocates k2).
  MM1 for h1 runs at PE tile position (64,0).
- The attention for (j0,h0) and (j0,h1) is interleaved chunk-by-chunk
  into phase A (1-chunk exp groups), so the scalar engine works from
  ~23us in. This also smooths the power ramp that previously drew a
  133us half-clock throttle window at the phase A->C transition.
- MM1 alternates even k-chunks onto PE row-group half 0:64 and odd
  chunks onto 64:128 (via partition-swapped copies of q^T/k^T built with
  two SBUF->SBUF DMAs per slice): consecutive MM1s never share a row
  group, so the PE reorder window pulls every LDWEIGHTS ahead of the
  in-flight matmul and adjacent MM1s partially overlap (248 -> 183 ns
  measured for N=512).
- MM2 groups are emitted up to TWO exp-groups late: in the strictly
  in-order PE queue, the next MM1 trio must never sit behind an MM2
  backlog waiting on exp, or the scalar (bottleneck) goes idle -- the
  depth-2 queue keeps head boundaries seamless. The same deferral runs
  continuously across heads and q-slices; the output projection is
  deferred by one q-slice.
- exp runs on ScalarE straight from PSUM [128,1536] (3 banks, double
  buffered; PSUM budget is exactly 8 banks with o_t + y PSUM) into bf16
  SBUF. Scores are O(2) by construction; max-subtraction is unnecessary.
- v_aug slots are 128 wide ([64 v | ones col | 63 zero pad]) so MM2's
  weight loads take the fast-weight-load path; the ones column
  accumulates softmax denominators in PSUM row 64 for free.
- normalize runs off the PE: the [1,512] denominator row is reshaped to
  [128,4] by DMA so the iterative-divide DVE reciprocal costs 32 cycles
  instead of 4096, then a DRAM round-trip DMA broadcasts 1/d across 64
  partitions (partition-stride-0 read) for one DVE multiply per head.

Rejected experiments (measured slower or too lossy on HW): fp8 DoubleRow
MM1 with k hi+lo expansion (LDWEIGHTS-bound, 545us and 1.8e-2 error),
PE row-tiling paired MM1 chunks (2nd tile streams +163ns late, no gain),
per-chunk prologue DMA splitting (HW race -> wrong results).
"""

import numpy as np
import ml_dtypes

import concourse.bass as bass
import concourse.mybir as mybir
import concourse.tile as tile
from concourse import bacc
from concourse.bass_utils import run_bass_kernel_spmd

F32 = mybir.dt.float32
BF16 = mybir.dt.bfloat16

DIM = 768
NUM_HEADS = 12
HEAD_DIM = 64
SCALE = HEAD_DIM ** -0.5
B = 2
N_FULL = 4096
N_CORES = 8
HEADS_PER_CORE = 3
CC = DIM // 128  # 6 contraction chunks


def build_nc(n=N_FULL, fast_mm=True):
    """Build the per-core Bass program. Same program runs SPMD on all
    cores; per-core inputs differ (x^T batch + per-head weight slices)."""
    del fast_mm
    nj = n // 512      # q slices
    nk = n // 128      # k chunks
    nc = bacc.Bacc("TRN2", target_bir_lowering=False, debug=False)

    # x^T and weights pre-tiled on host to partition-major layouts so DMA
    # lines are contiguous multi-KB runs per partition.
    xt_d = nc.dram_tensor("xt", [128, nj, CC, 512], BF16, kind="ExternalInput")
    wqk_d = nc.dram_tensor("wqk", [128, CC, 384], BF16, kind="ExternalInput")
    wv_d = nc.dram_tensor("wv", [128, CC, 192], BF16, kind="ExternalInput")
    wp_d = nc.dram_tensor("wp", [192, DIM], BF16, kind="ExternalInput")
    # partial y^T in bf16: host sums the four per-core partials in fp32
    yt_d = nc.dram_tensor("yt", [DIM, n], BF16, kind="ExternalOutput")

    # MM1/exp/MM2 group sizes: 3 k-chunks (3 PSUM banks) per exp.
    groups = [3] * (nk // 3)
    if nk % 3:
        groups.append(nk % 3)

    lp = nc.allow_low_precision(
        reason="bf16 matmul operands; PSUM accumulation stays fp32")
    with lp, tile.TileContext(nc) as tc:
        consts = tc.alloc_tile_pool(name="consts", bufs=1)
        persist = tc.alloc_tile_pool(name="persist", bufs=1)

        wqk_sb = [consts.tile([128, 384], BF16, tag=f"wqk{cc}", name=f"wqk{cc}")
                  for cc in range(CC)]
        wv_sb = consts.tile([128, CC, 192], BF16, tag="wv")

        # Preload the exp table set (~2.7us) off the critical path.
        warm = consts.tile([1, 4], F32, tag="warm")
        nc.vector.memset(warm, 0.0)
        nc.scalar.activation(out=warm, in_=warm,
                             func=mybir.ActivationFunctionType.Exp)

        # Persistent activations. Partition homes: h0 0:64, h1 64:128,
        # h2 0:64; the *x tensors hold the same data with the partition
        # halves swapped. MM1 sends even k-chunks to one PE row-group half
        # and odd chunks to the other: consecutive matmuls then never
        # share a row group, so the PE's reorder window can pull each
        # LDWEIGHTS ahead of the in-flight matmul (~35ns/matmul).
        qt01 = persist.tile([128, n], BF16, tag="qt01")
        qt01x = persist.tile([128, n], BF16, tag="qt01x")
        kt01 = persist.tile([128, nk, 128], BF16, tag="kt01")
        kt01x = persist.tile([128, nk, 128], BF16, tag="kt01x")
        qt2 = persist.tile([128, n], BF16, tag="qt2")
        kt2 = persist.tile([128, nk, 128], BF16, tag="kt2")
        # v_aug slot: [64 v dims | ones | 63 zero pad] -> 128-wide weights
        # keep MM2 on the fast-weight-load path.
        v_aug = persist.tile([128, HEADS_PER_CORE, nk, 128], BF16, tag="vaug")
        nc.vector.memset(v_aug[:, :, :, 64:65], 1.0)
        nc.vector.memset(v_aug[:, :, :, 65:128], 0.0)

        # Pools shared by the overlapped phase-A attention prefix and
        # phase C. PSUM budget: o_ps 2 banks + phase A (qk 2, v 2, s_mini 2)
        # = 8; after phase A closes, s_ps (6 banks) + o_ps = 8.
        wp_sb = {}
        with (
            tc.tile_pool(name="o_ps", bufs=2, space="PSUM") as o_ps,
            tc.tile_pool(name="dscr", bufs=3, space="DRAM") as dscr_p,
            tc.tile_pool(name="ptp", bufs=6) as ptp,
            tc.tile_pool(name="otp", bufs=4) as otp,
            tc.tile_pool(name="rsbp", bufs=3) as rsbp,
            tc.tile_pool(name="ytp", bufs=4) as ytp,
        ):
            ots_by_j = {}
            pending_proj = None
            state = {"pend": []}
            o_ts = {}

            def emit_outproj(pj, pots):
                # pots = [ot01 (parts 0:64 h0, 64:128 h1), ot2]
                pjsl = bass.ts(pj, 512)
                ot01, ot2 = pots
                for cc in range(CC):
                    yps = o_ps.tile([128, 512], F32, tag="o", name="yps")
                    nc.tensor.matmul(
                        yps, wp_sb["01"][:, bass.ts(cc, 128)], ot01,
                        start=True, stop=False,
                    )
                    nc.tensor.matmul(
                        yps, wp_sb["2"][:, bass.ts(cc, 128)], ot2,
                        start=False, stop=True,
                    )
                    yst = ytp.tile([128, 512], BF16, tag="yt")
                    nc.vector.tensor_copy(out=yst, in_=yps)
                    nc.sync.dma_start(out=yt_d[bass.ts(cc, 128), pjsl], in_=yst)

            def normalize(pj, ph, o_t):
                nonlocal pending_proj
                # denominators: [1,512] PSUM row -> [128,4] via DMA so the
                # 8-cycle/element reciprocal runs 32 cycles, then a DRAM
                # round trip broadcasts 1/d to 64 partitions.
                rsb = rsbp.tile([128, 512], F32, tag="r")
                nc.vector.tensor_copy(out=rsb[64:65, :], in_=o_t[64:65, :])
                d4 = rsbp.tile([128, 4], F32, tag="d4")
                nc.sync.dma_start(out=d4, in_=rsb[64:65, :])
                r4 = rsbp.tile([128, 4], F32, tag="r4")
                nc.vector.reciprocal(out=r4, in_=d4)
                scr = dscr_p.tile([512], F32, tag="scr")
                nc.sync.dma_start(
                    out=scr.rearrange("(p i) -> p i", p=128), in_=r4)
                bcs = rsbp.tile([64, 512], F32, tag="bcs")
                scr_b = bass.AP(tensor=scr.tensor, offset=scr.offset,
                                ap=[[0, 64]] + list(scr.ap))
                nc.sync.dma_start(out=bcs, in_=scr_b)
                ots = ots_by_j.setdefault(pj, [])
                if ph == 0:
                    ot01 = otp.tile([128, 512], BF16, tag="ot01", name="ot01")
                    nc.vector.tensor_mul(ot01[0:64, :], o_t[0:64, :], bcs)
                    ots.append(ot01)
                    if pending_proj is not None:
                        emit_outproj(*pending_proj)
                        pending_proj = None
                elif ph == 1:
                    ot1 = otp.tile([64, 512], BF16, tag="ot1", name="ot1")
                    nc.vector.tensor_mul(ot1, o_t[0:64, :], bcs)
                    nc.sync.dma_start(out=ots[0][64:128, :], in_=ot1)
                else:
                    ot2 = otp.tile([64, 512], BF16, tag="ot2", name="ot2")
                    nc.vector.tensor_mul(ot2, o_t[0:64, :], bcs)
                    ots.append(ot2)
                    pending_proj = (pj, ots)

            def flush_mm2():
                pj, ph, ks_p, gs_p, ptt_p = state["pend"].pop(0)
                key = (pj, ph)
                if key not in o_ts:
                    o_ts[key] = o_ps.tile([128, 512], F32, tag="o", name="o_t")
                o_t = o_ts[key]
                for t in range(gs_p):
                    kc = ks_p + t
                    nc.tensor.matmul(
                        o_t, v_aug[:, ph, kc, :],
                        ptt_p[:, bass.ts(t, 512)],
                        start=(kc == 0), stop=(kc == nk - 1),
                    )
                if ks_p + gs_p == nk:  # head complete
                    del o_ts[key]
                    normalize(pj, ph, o_t)

            def mm1_operands(pj, ph, kc):
                pjsl = bass.ts(pj, 512)
                if ph == 0:
                    if kc % 2 == 0:
                        return kt01[0:64, kc, :], qt01[0:64, pjsl]
                    return kt01x[64:128, kc, :], qt01x[64:128, pjsl]
                if ph == 1:
                    if kc % 2 == 0:
                        return kt01[64:128, kc, :], qt01[64:128, pjsl]
                    return kt01x[0:64, kc, :], qt01x[0:64, pjsl]
                if kc % 2 == 0:
                    return kt2[0:64, kc, :], qt2[0:64, pjsl]
                return kt2[64:128, kc, :], qt2[64:128, pjsl]

            # ---- Phase A: QKV projections, with the attention for
            # (j0,h0)/(j0,h1) interleaved chunk-by-chunk so the scalar
            # engine (the kernel bottleneck) is busy from the start ----
            with (
                tc.tile_pool(name="xtj", bufs=3) as xtj_p,
                tc.tile_pool(name="stage", bufs=2) as stage_p,
                tc.tile_pool(name="qk_ps", bufs=2, space="PSUM") as qk_ps,
                tc.tile_pool(name="v_ps", bufs=2, space="PSUM") as v_ps,
                tc.tile_pool(name="s_mini", bufs=2, space="PSUM") as s_mini,
            ):
                for j in range(nj):
                    jsl = bass.ts(j, 512)
                    if j == 0:
                        xtj_cc = [xtj_p.tile([128, 512], BF16, tag=f"xt0_{cc}",
                                             name=f"xt0_{cc}")
                                  for cc in range(CC)]
                        for cc in range(CC):
                            nc.sync.dma_start(out=xtj_cc[cc],
                                              in_=xt_d[:, 0, cc, :])
                    else:
                        xtj = xtj_p.tile([128, CC, 512], BF16, tag="xtj")
                        nc.sync.dma_start(out=xtj, in_=xt_d[:, j])
                        xtj_cc = [xtj[:, cc, :] for cc in range(CC)]
                    if j == 0:
                        for cc in range(CC):
                            nc.sync.dma_start(out=wqk_sb[cc],
                                              in_=wqk_d[:, cc, :])
                        nc.sync.dma_start(out=wv_sb, in_=wv_d[:, :, :])
                    # q/k projections: [q0|q1], [k0|k1], [q2|k2]
                    for pi, colbase in enumerate((0, 128, 256)):
                        ps = qk_ps.tile([128, 512], F32, tag="qk")
                        for cc in range(CC):
                            nc.tensor.matmul(
                                ps,
                                wqk_sb[cc][:, colbase:colbase + 128],
                                xtj_cc[cc],
                                start=(cc == 0), stop=(cc == CC - 1),
                            )
                        if pi == 0:
                            nc.vector.tensor_copy(out=qt01[:, jsl], in_=ps)
                            nc.sync.dma_start(out=qt01x[64:128, jsl],
                                              in_=qt01[0:64, jsl])
                            nc.sync.dma_start(out=qt01x[0:64, jsl],
                                              in_=qt01[64:128, jsl])
                        elif pi == 1:
                            nc.vector.tensor_copy(
                                out=kt01[:, 4 * j:4 * j + 4, :], in_=ps)
                            nc.sync.dma_start(
                                out=kt01x[64:128, 4 * j:4 * j + 4, :],
                                in_=kt01[0:64, 4 * j:4 * j + 4, :])
                            nc.sync.dma_start(
                                out=kt01x[0:64, 4 * j:4 * j + 4, :],
                                in_=kt01[64:128, 4 * j:4 * j + 4, :])
                        else:
                            # q2 -> partitions 0:64 + DMA-duplicated to the
                            # upper half; k2 odd chunks land at 64:128
                            # directly, even chunks relocate down via DMA
                            nc.vector.tensor_copy(out=qt2[0:64, jsl], in_=ps[0:64, :])
                            nc.sync.dma_start(out=qt2[64:128, jsl],
                                              in_=qt2[0:64, jsl])
                            nc.vector.tensor_copy(
                                out=kt2[64:128, 4 * j + 1, :],
                                in_=ps[64:128, 128:256])
                            nc.vector.tensor_copy(
                                out=kt2[64:128, 4 * j + 3, :],
                                in_=ps[64:128, 384:512])
                            st = stage_p.tile([128, 512], BF16, tag="st")
                            nc.vector.tensor_copy(out=st[64:128, 0:128],
                                                  in_=ps[64:128, 0:128])
                            nc.vector.tensor_copy(out=st[64:128, 128:256],
                                                  in_=ps[64:128, 256:384])
                            nc.sync.dma_start(out=kt2[0:64, 4 * j, :],
                                              in_=st[64:128, 0:128])
                            nc.sync.dma_start(out=kt2[0:64, 4 * j + 2, :],
                                              in_=st[64:128, 128:256])

                    # v projection (natural orientation), 3 heads packed
                    for rc in range(4):
                        psv = v_ps.tile([128, 192], F32, tag="v")
                        for cc in range(CC):
                            nc.tensor.matmul(
                                psv,
                                xtj_cc[cc][:, bass.ts(rc, 128)],
                                wv_sb[:, cc, :],
                                start=(cc == 0), stop=(cc == CC - 1),
                            )
                        kc = j * 4 + rc
                        for h in range(HEADS_PER_CORE):
                            nc.vector.tensor_copy(
                                out=v_aug[:, h, kc, 0:64], in_=psv[:, bass.ts(h, 64)]
                            )

                    # attention prefix: chunks 4j..4j+3 of (j0,h0), (j0,h1)
                    for c in range(4 * j, 4 * j + 4):
                        for h in (0, 1):
                            kt_sl, q_sl = mm1_operands(0, h, c)
                            sp = s_mini.tile([128, 512], F32, tag="sm")
                            nc.tensor.matmul(sp, kt_sl, q_sl,
                                             start=True, stop=True)
                            ptt = ptp.tile([128, 512], BF16, tag="pt1",
                                           name="pt1")
                            nc.scalar.activation(
                                out=ptt, in_=sp,
                                func=mybir.ActivationFunctionType.Exp,
                            )
                            if len(state["pend"]) >= 2:
                                flush_mm2()
                            state["pend"].append((0, h, c, 1, ptt))

            # out-projection weights are first needed one q-slice into
            # phase C; loading them late keeps the prologue DMA clear.
            wp_sb["01"] = consts.tile([128, DIM], BF16, tag="wp01", name="wp01")
            nc.sync.dma_start(out=wp_sb["01"], in_=wp_d[0:128, :])
            wp_sb["2"] = consts.tile([64, DIM], BF16, tag="wp2", name="wp2")
            nc.sync.dma_start(out=wp_sb["2"], in_=wp_d[128:192, :])

            # ---- Phase C/D: remaining heads + output projection ----
            head_list = [(0, 2)] + [(j, h) for j in range(1, nj)
                         for h in range(HEADS_PER_CORE)]
            with tc.tile_pool(name="s_ps", bufs=2, space="PSUM") as s_ps:
                for j, h in head_list:
                    ks = 0
                    for gsize in groups:
                        sp = s_ps.tile([128, 1536], F32, tag="s")
                        for t in range(gsize):
                            kt_sl, q_sl = mm1_operands(j, h, ks + t)
                            nc.tensor.matmul(
                                sp[:, bass.ts(t, 512)], kt_sl,
                                q_sl, start=True, stop=True,
                            )
                        ptt = ptp.tile([128, 1536], BF16, tag="pt")
                        nc.scalar.activation(
                            out=ptt[:, 0:gsize * 512], in_=sp[:, 0:gsize * 512],
                            func=mybir.ActivationFunctionType.Exp,
                        )
                        # MM2 up to two groups behind: in the in-order PE
                        # queue, exp(g) must never wait on a trio that sits
                        # behind an MM2 backlog.
                        if len(state["pend"]) >= 2:
                            flush_mm2()
                        state["pend"].append((j, h, ks, gsize, ptt))
                        ks += gsize
                while state["pend"]:
                    flush_mm2()
                emit_outproj(*pending_proj)

        persist.release()
        consts.release()

    nc.compile()
    return nc


def make_core_inputs(x_b, w_qkv, w_proj, hbase):
    """Per-core input arrays for heads [hbase, hbase+3) of batch x_b."""
    C = DIM
    bf16 = ml_dtypes.bfloat16
    wq = [w_qkv[(hbase + h) * 64:(hbase + h + 1) * 64, :] * SCALE for h in range(3)]
    wk = [w_qkv[C + (hbase + h) * 64:C + (hbase + h + 1) * 64, :] for h in range(3)]
    wv = [w_qkv[2 * C + (hbase + h) * 64:2 * C + (hbase + h + 1) * 64, :] for h in range(3)]

    wqk = np.zeros((C, 384), np.float32)
    wqk[:, 0:64] = wq[0].T
    wqk[:, 64:128] = wq[1].T
    wqk[:, 128:192] = wk[0].T
    wqk[:, 192:256] = wk[1].T
    wqk[:, 256:320] = wq[2].T
    wqk[:, 320:384] = wk[2].T
    # pre-tile [768, m] -> [p, a, m] (row a*128+p) for contiguous DMA lines
    wqk = np.ascontiguousarray(wqk.reshape(CC, 128, 384).transpose(1, 0, 2))

    wv_p = np.zeros((C, 192), np.float32)
    for h in range(3):
        wv_p[:, h * 64:(h + 1) * 64] = wv[h].T
    wv_p = np.ascontiguousarray(wv_p.reshape(CC, 128, 192).transpose(1, 0, 2))

    wp = np.zeros((192, C), np.float32)
    for h in range(3):
        wp[h * 64:(h + 1) * 64, :] = w_proj[:, (hbase + h) * 64:(hbase + h + 1) * 64].T

    # x^T [768, n] -> [p, j, a, m] tiling: row a*128+p, col j*512+m
    n = x_b.shape[0]
    xt = np.ascontiguousarray(
        x_b.T.reshape(CC, 128, n // 512, 512).transpose(1, 2, 0, 3))
    return {
        "xt": xt.astype(bf16),
        "wqk": wqk.astype(bf16),
        "wv": wv_p.astype(bf16),
        "wp": wp.astype(bf16),
    }


_NC_CACHE = {}


def get_nc(n=N_FULL, fast_mm=True):
    key = (n, fast_mm)
    if key not in _NC_CACHE:
        _NC_CACHE[key] = build_nc(n, fast_mm)
    return _NC_CACHE[key]


def kernel(x, w_qkv, w_proj, b_proj, _trace=False):
    x = np.asarray(x, np.float32)
    w_qkv = np.asarray(w_qkv, np.float32)
    w_proj = np.asarray(w_proj, np.float32)
    b_proj = np.asarray(b_proj, np.float32)

    nc = get_nc(N_FULL, True)
    in_maps = []
    for c in range(N_CORES):
        b = c // 4
        hbase = (c % 4) * HEADS_PER_CORE
        in_maps.append(make_core_inputs(x[b], w_qkv, w_proj, hbase))

    res = run_bass_kernel_spmd(nc, in_maps, core_ids=list(range(N_CORES)),
                               trace=_trace)
    y = np.empty((B, N_FULL, DIM), np.float32)
    for b in range(B):
        acc = res.results[4 * b]["yt"].astype(np.float32)
        for c in range(4 * b + 1, 4 * b + 4):
            acc = acc + res.results[c]["yt"]
        y[b] = acc.T + b_proj[None, :]
    if _trace:
        return y, res
    return y
